# revision 5
# baseline (speedup 1.0000x reference)
"""Trainium2 Bass kernel for the DJconv hypergraph message-passing layer.

Reference computation (per full input):
    gram = H.T @ H                              [E, E]
    Hu   = concat([H, H @ gram], 1) >= 0.5      [N, 2E] binary
    dv   = Hu.sum(1);  inv = rsqrt(dv) (0 where dv==0)
    out  = ((1 + inv)[:, None] * U) @ weight + bias

Sharding: rows (nodes) split across 8 NeuronCores; the [E, E] gram partial
is AllReduced (bf16, exact for the 0-vs-nonzero threshold decisions);
weight/bias replicated.

v3 structure:
 - H streamed as fp8e4 (exact 0/1): DoubleRow gram, fp8 PE transposes
   (element-step-2 PSUM) feeding phase-B DoubleRow weights.
 - per-node H rowsums (dvH) computed during phase A on DVE, so phase B has
   no ones-column and no per-tile accumulator copies.
 - U fully prebuffered in bf16 so its DMA issues precede the collective
   trigger on the in-order gpsimd stream; U^T + U@W (bf16) precomputed
   into SBUF during the collective window.
 - post-collective: DoubleRow HG matmuls + thresholds (DVE/ACT split) +
   one fused scale+bias op per tile, output streamed per supertile.
"""

import numpy as np
import ml_dtypes

import concourse.bass as bass
import concourse.tile as tile
from concourse import bacc, mybir
from concourse.bass_utils import run_bass_kernel_spmd

F32 = mybir.dt.float32
BF16 = mybir.dt.bfloat16
FP8 = mybir.dt.float8e4

N_FULL, E, IN_C, OUT_C = 131072, 256, 128, 256
NCORES = 8
ROWS = N_FULL // NCORES          # 16384 rows per core
P = 128
SCALE = 64.0
GW = E  # phase-B matmul width (no ones column needed)


def build_program(rows=ROWS, ncores=NCORES):
    assert rows % 512 == 0
    nt = rows // P          # node tiles per core
    ns = nt // 4            # supertiles (4 node tiles each)

    nc = bacc.Bacc("TRN2", target_bir_lowering=False, debug=False,
                   num_devices=ncores)

    H = nc.dram_tensor("H", [rows, E], F32, kind="ExternalInput").ap()
    U = nc.dram_tensor("U", [rows, IN_C], F32, kind="ExternalInput").ap()
    W16 = nc.dram_tensor("W16", [IN_C, OUT_C], BF16, kind="ExternalInput").ap()
    B16 = nc.dram_tensor("B16", [P, OUT_C], BF16, kind="ExternalInput").ap()
    B32 = nc.dram_tensor("B32", [P, OUT_C], F32, kind="ExternalInput").ap()
    ID8 = nc.dram_tensor("ID8", [P, P], FP8, kind="ExternalInput").ap()
    ID16 = nc.dram_tensor("ID16", [P, P], BF16, kind="ExternalInput").ap()
    OUT = nc.dram_tensor("OUT", [rows, OUT_C], F32, kind="ExternalOutput").ap()

    # node (s*512 + p*4 + j): 4 consecutive rows per partition -> 4KB DMA lines.
    # Same permutation applied to H, U and OUT, so the kernel is self-consistent.
    H_r = H.rearrange("(s p j) e -> s p j e", j=4, p=P)
    U_r = U.rearrange("(s p j) c -> s p j c", j=4, p=P)
    OUT_r = OUT.rearrange("(s p j) o -> s p j o", j=4, p=P)

    with tile.TileContext(nc) as tc:
        _body(tc, nt, ns, H_r, U_r, OUT_r, W16, B16, B32, ID8, ID16)

    nc.compile()
    return nc


def _body(tc, nt, ns, H_r, U_r, OUT_r, W16, B16, B32, ID8, ID16):
    nc = tc.nc
    Add = mybir.AluOpType.add
    Mult = mybir.AluOpType.mult
    IsGe = mybir.AluOpType.is_ge
    AF = mybir.ActivationFunctionType
    DR = mybir.MatmulPerfMode.DoubleRow
    AX = mybir.AxisListType.X

    import contextlib
    ctx = contextlib.ExitStack()
    with ctx:
        const = ctx.enter_context(tc.tile_pool(name="const", bufs=1))
        htst = ctx.enter_context(tc.tile_pool(name="htstore", bufs=1))
        work = ctx.enter_context(tc.tile_pool(name="work", bufs=1))
        ob16p = ctx.enter_context(tc.tile_pool(name="ob16", bufs=3))
        ob32p = ctx.enter_context(tc.tile_pool(name="ob32", bufs=3))
        scr = ctx.enter_context(tc.tile_pool(name="scratch", bufs=3))
        dram = ctx.enter_context(tc.tile_pool(name="dram", bufs=1, space="DRAM"))

        # ---- constants ----
        id8 = const.tile([P, P], FP8)
        nc.sync.dma_start(id8[:], ID8[:])
        id16 = const.tile([P, P], BF16)
        nc.sync.dma_start(id16[:], ID16[:])
        w_sb = const.tile([IN_C, OUT_C], BF16)
        nc.sync.dma_start(w_sb[:], W16[:])
        bias16 = const.tile([P, OUT_C], BF16)
        nc.sync.dma_start(bias16[:], B16[:])
        bias32 = const.tile([P, OUT_C], F32)
        nc.sync.dma_start(bias32[:], B32[:])
        neghalf = const.tile([P, 1], F32)
        nc.vector.memset(neghalf[:], -0.5 / SCALE)

        # persistent H^T (fp8, DR-packed: slot t holds edges t*128..t*128+127)
        HTE = htst.tile([P, 2, nt * P], FP8, tag="hte")
        # U prebuffer (bf16) so all U DMA issues precede the collective trigger
        U16 = htst.tile([P, ns, 4, IN_C], BF16, tag="u16")

        dvH = work.tile([P, nt], F32, tag="dvH")   # rowsum(H) per node
        gcat = work.tile([P, E + P], BF16, tag="gcat")

        # ---- phase A: stream H as fp8; DR gram + fp8 transposes chase DMA ----
        with tc.tile_pool(name="hallp", bufs=1) as hallp:
            H8 = hallp.tile([P, ns, 4, E], FP8, tag="h8")
            with tc.tile_pool(name="psA", bufs=1, space="PSUM") as psA, \
                 tc.tile_pool(name="psT", bufs=3, space="PSUM") as psT:
                gA = psA.tile([P, E], F32, tag="gA")
                gB = psA.tile([P, P], F32, tag="gB")
                for s in range(ns):
                    nc.gpsimd.dma_start(H8[:, s], H_r[s])   # f32 -> fp8 cast
                    for q in (0, 2):
                        first = (s == 0 and q == 0)
                        last = (s == ns - 1 and q == 2)
                        nc.tensor.matmul(gA[:], H8[:, s, q:q + 2, 0:P],
                                         H8[:, s, q:q + 2, :],
                                         perf_mode=DR, start=first, stop=last)
                        nc.tensor.matmul(gB[:], H8[:, s, q:q + 2, P:E],
                                         H8[:, s, q:q + 2, P:E],
                                         perf_mode=DR, start=first, stop=last)
                    pt0 = psT.tile([P, 4, P, 2], FP8, tag="t0")
                    pt1 = psT.tile([P, 4, P, 2], FP8, tag="t1")
                    for j in range(4):
                        nc.tensor.transpose(pt0[:, j, :, 0], H8[:, s, j, 0:P],
                                            id8[:])
                        nc.tensor.transpose(pt1[:, j, :, 0], H8[:, s, j, P:E],
                                            id8[:])
                    sl = slice(s * 4 * P, (s + 1) * 4 * P)
                    # HT copies on ACT; rowsums on DVE — keeps both streams
                    # clear of the gram->collective critical path
                    nc.scalar.copy(HTE[:, 0, sl], pt0[:, :, :, 0])
                    nc.scalar.copy(HTE[:, 1, sl], pt1[:, :, :, 0])
                    nc.vector.tensor_reduce(dvH[:, 4 * s:4 * s + 4], H8[:, s],
                                            axis=AX, op=Add)
                # gcat on ACT: DVE has a rowsum backlog at this point
                nc.scalar.copy(gcat[:, 0:E], gA[:])
                nc.scalar.copy(gcat[:, E:E + P], gB[:])

        # ---- U DMA issues (gpsimd stream, before the collective trigger) ----
        for s in range(ns):
            nc.gpsimd.dma_start(U16[:, s], U_r[s])      # f32 -> bf16 cast

        # ---- collective: AllReduce the bf16 gram partial ----
        cc_in = dram.tile([P, E + P], BF16)
        cc_out = dram.tile([P, E + P], BF16)
        nc.sync.dma_start(cc_in[:], gcat[:])
        nc.gpsimd.collective_compute(
            "AllReduce", Add,
            replica_groups=[list(range(NCORES))],
            ins=[cc_in.opt()],
            outs=[cc_out.opt()],
        )
        gsum = work.tile([P, E + P], BF16, tag="gsum")
        nc.sync.dma_start(gsum[:], cc_out[:])

        # ---- U^T + U@W precompute in bf16 (fills the collective window) ----
        uwp = ctx.enter_context(tc.tile_pool(name="uwp", bufs=1))
        UW = uwp.tile([P, nt, OUT_C], BF16, tag="uw")
        with tc.tile_pool(name="utring", bufs=3) as utring, \
             tc.tile_pool(name="psU", bufs=3, space="PSUM") as psU, \
             tc.tile_pool(name="psF", bufs=4, space="PSUM") as psF:
            for s in range(ns):
                pp = psU.tile([P, 4, IN_C], BF16, tag="pp")
                for j in range(4):
                    nc.tensor.transpose(pp[:, j], U16[:, s, j], id16[:])
                utr = utring.tile([P, 4, IN_C], BF16, tag="ut")
                if s % 2 == 0:
                    nc.vector.tensor_copy(utr[:], pp[:])
                else:
                    nc.scalar.copy(utr[:], pp[:])
                for j in range(4):
                    k = 4 * s + j
                    po = psF.tile([P, OUT_C], F32, tag="po")
                    nc.tensor.matmul(po[:], utr[:, j], w_sb[:],
                                     start=True, stop=True)
                    if k % 2 == 0:
                        nc.vector.tensor_copy(UW[:, k], po[:])
                    else:
                        nc.scalar.copy(UW[:, k], po[:])

        # ---- gxp: fp8 DR-packed gram (scaled 1/SCALE) ----
        gxp = const.tile([P, 2, GW], FP8, tag="gxp")
        with tc.tile_pool(name="psG", bufs=1, space="PSUM") as psG:
            nc.vector.tensor_scalar(gxp[:, 0, :], gsum[:, 0:E], 1.0 / SCALE,
                                    None, op0=Mult)
            nc.vector.tensor_scalar(gxp[:, 1, P:E], gsum[:, E:E + P],
                                    1.0 / SCALE, None, op0=Mult)
            pgt = psG.tile([P, P], BF16, tag="pgt")
            nc.tensor.transpose(pgt[:], gsum[:, P:E], id16[:])
            nc.vector.tensor_scalar(gxp[:, 1, 0:P], pgt[:], 1.0 / SCALE,
                                    None, op0=Mult)

        # ---- phase B + final epilogue, interleaved in chunks of 32 tiles ----
        dvS = work.tile([P, nt], F32, tag="dvS")
        s1p = work.tile([P, nt], F32, tag="s1p")     # 1 + rsqrt(dv)
        CH = 32
        NACT = 12  # tiles per chunk thresholded on ACT (contiguous tail)

        def dv_chunk(c0, c1):
            csl = slice(c0, c1)
            # ACT-thresholded tail holds 2*cnt-256 (Sign); fix to cnt
            nc.vector.tensor_scalar(dvS[:, c1 - NACT:c1], dvS[:, c1 - NACT:c1],
                                    0.5, float(E) / 2, op0=Mult, op1=Add)
            dv = work.tile([P, nt], F32, tag="dv")
            nc.vector.tensor_tensor(dv[:, csl], dvS[:, csl], dvH[:, csl], op=Add)
            mx = work.tile([P, nt], F32, tag="mx")
            nc.vector.tensor_scalar_max(mx[:, csl], dv[:, csl], 1.0)
            rc = work.tile([P, nt], F32, tag="rc")
            nc.vector.reciprocal(rc[:, csl], mx[:, csl])
            sq = work.tile([P, nt], F32, tag="sq")
            nc.scalar.sqrt(sq[:, csl], dv[:, csl])
            # s1p = 1 + sqrt(dv)/max(dv,1)  (== 1 + rsqrt(dv), 1 where dv==0)
            r0 = work.tile([P, nt], F32, tag="r0")
            nc.vector.tensor_tensor(r0[:, csl], sq[:, csl], rc[:, csl], op=Mult)
            nc.vector.tensor_scalar_add(s1p[:, csl], r0[:, csl], 1.0)

        with tc.tile_pool(name="psB", bufs=6, space="PSUM") as psB:
            for c0 in range(0, nt, CH):
                for k in range(c0, c0 + CH):
                    pb = psB.tile([P, GW], F32, tag="pb")
                    ksl = slice(k * P, (k + 1) * P)
                    nc.tensor.matmul(pb[:], HTE[:, :, ksl], gxp[:],
                                     perf_mode=DR, start=True, stop=True)
                    sg = scr.tile([P, E], BF16, tag="sg")
                    if k - c0 < CH - NACT:
                        nc.vector.tensor_scalar(sg[:], pb[:], 0.5 / SCALE,
                                                0.0, op0=IsGe, op1=Add,
                                                accum_out=dvS[:, k:k + 1])
                    else:
                        nc.scalar.activation(sg[:], pb[:], AF.Sign,
                                             bias=neghalf[:], scale=1.0,
                                             accum_out=dvS[:, k:k + 1])
                dv_chunk(c0, c0 + CH)
                for s in range(c0 // 4, (c0 + CH) // 4):
                    # ys = UW*s1p (DVE tensor_scalar, 4x); ob = ys + bias
                    # (tensor_tensor 2x, DVE/Pool alternating); casting OUT DMA
                    ob = ob16p.tile([P, 4, OUT_C], BF16, tag="ob16")
                    for j in range(4):
                        k = 4 * s + j
                        ys = scr.tile([P, OUT_C], BF16, tag="ys")
                        nc.vector.tensor_scalar(ys[:], UW[:, k],
                                                s1p[:, k:k + 1], None, op0=Mult)
                        if k % 2 == 0:
                            nc.vector.tensor_tensor(ob[:, j], ys[:], bias16[:],
                                                    op=Add)
                        else:
                            nc.gpsimd.tensor_tensor(ob[:, j], ys[:], bias16[:],
                                                    op=Add)
                    nc.gpsimd.dma_start(OUT_r[s], ob[:])  # bf16 -> f32


_CACHE = {}


def _get_program(rows=ROWS):
    if rows not in _CACHE:
        _CACHE[rows] = build_program(rows=rows)
    return _CACHE[rows]


def kernel(H, U, weight, bias, _rows=ROWS, _trace=False):
    H = np.ascontiguousarray(H, dtype=np.float32)
    U = np.ascontiguousarray(U, dtype=np.float32)
    w16 = np.ascontiguousarray(weight, dtype=np.float32).astype(ml_dtypes.bfloat16)
    bias32 = np.broadcast_to(
        np.ascontiguousarray(bias, dtype=np.float32).reshape(1, OUT_C), (P, OUT_C)
    ).copy()
    bias16 = bias32.astype(ml_dtypes.bfloat16)
    id8 = np.eye(P, dtype=mybir.dt.np(FP8))
    id16 = np.eye(P, dtype=ml_dtypes.bfloat16)

    nc = _get_program(_rows)
    in_maps = []
    for i in range(NCORES):
        sl = slice(i * _rows, (i + 1) * _rows)
        in_maps.append({
            "H": H[sl], "U": U[sl], "W16": w16, "B16": bias16, "B32": bias32,
            "ID8": id8, "ID16": id16,
        })
    res = run_bass_kernel_spmd(nc, in_maps, core_ids=list(range(NCORES)),
                               trace=_trace)
    out = np.concatenate([res.results[i]["OUT"] for i in range(NCORES)], axis=0)
    if _trace:
        return out, res
    return out
